# revision 1
# baseline (speedup 1.0000x reference)
"""Trainium2 Bass kernel for nn_MultiHeadAttention_56375740727430.

Causal multi-head attention, B=2 S=2048 D=1024 H=16 KS=64, followed by an
output projection `heads @ kernel`.

Sharding: pure data/head parallel over 8 cores — core c handles batch c//4
and 4 heads (c%4)*4 ... +4.  Each core computes Q^T/K^T (head-pair-stacked,
transposed layout), V (natural layout, with an appended ones-column so the
softmax denominator Z falls out of the attention matmul), causal scores ->
exp -> (P@V | Z) -> per-head output projection, all unnormalized.  The host
divides by Z, sums head contributions and batch-partials, and transposes.

Matmul operands are bf16 (1 cycle/row on the PE); accumulation, scores (exp
input) and Z stay fp32.
"""

import sys

sys.path.insert(0, "/opt/trn_rl_repo")

from contextlib import ExitStack

import ml_dtypes
import numpy as np

import concourse.bass as bass
import concourse.bacc as bacc
import concourse.mybir as mybir
import concourse.tile as tile

B, S, D = 2, 2048, 1024
H, KS = 16, 64

P = 128            # partitions
NCORES = 8
CORES_PER_B = NCORES // B          # 4
NH = H // CORES_PER_B              # heads per core = 4
NW = NH * KS                       # per-core projection width = 256
DT = D // P                        # d-tiles = 8
ST = S // P                        # s/l-tiles = 16
IB = 512                           # query block
NIB = S // IB                      # 4
LPB = IB // P                      # l-tiles per query block = 4

F32 = mybir.dt.float32
BF16 = mybir.dt.bfloat16
NP_BF16 = ml_dtypes.bfloat16
EXP = mybir.ActivationFunctionType.Exp


def build_nc():
    mm_dt = BF16
    nc = bacc.Bacc()

    xT = nc.declare_dram_parameter("xT", [D, S], mm_dt, isOutput=False)
    wq = nc.declare_dram_parameter("wq", [D, NW], mm_dt, isOutput=False)
    wk = nc.declare_dram_parameter("wk", [D, NW], mm_dt, isOutput=False)
    wv = nc.declare_dram_parameter("wv", [D, NW], mm_dt, isOutput=False)
    wkern = nc.declare_dram_parameter("wkern", [NH, KS, KS], mm_dt, isOutput=False)
    masks = nc.declare_dram_parameter("masks", [P, P], mm_dt, isOutput=False)
    outT = nc.declare_dram_parameter("outT", [NH, KS, S], F32, isOutput=True)
    z = nc.declare_dram_parameter("z", [NH, S], F32, isOutput=True)

    with tile.TileContext(nc) as tc, ExitStack() as ctx:
        const_pool = ctx.enter_context(tc.tile_pool(name="const", bufs=1))
        qkv_pool = ctx.enter_context(tc.tile_pool(name="qkv", bufs=1))
        out_pool = ctx.enter_context(tc.tile_pool(name="outp", bufs=1))
        xw_pool = ctx.enter_context(tc.tile_pool(name="xw", bufs=1))
        pexp_pool = ctx.enter_context(tc.tile_pool(name="pexp", bufs=4))
        osb_pool = ctx.enter_context(tc.tile_pool(name="osb", bufs=6))
        wkern_sb = const_pool.tile([KS, NH, KS], mm_dt)
        nc.sync.dma_start(wkern_sb[:], wkern[:].rearrange("h k j -> k h j"))
        mask_sb = const_pool.tile([P, P], mm_dt)
        nc.sync.dma_start(mask_sb[:], masks[:])

        qt_sb = [
            qkv_pool.tile([P, S], mm_dt, tag=f"qt{i}", name=f"qt{i}") for i in range(2)
        ]
        kt_sb = [
            qkv_pool.tile([P, S], mm_dt, tag=f"kt{i}", name=f"kt{i}") for i in range(2)
        ]
        v_sb = qkv_pool.tile([P, ST, NH, KS + 1], mm_dt, tag="v")
        nc.vector.memset(v_sb[:, :, :, KS], 1.0)
        outT_sb = out_pool.tile([KS, NH, S], F32)

        w_sb = {}
        for name, wh in (("q", wq), ("k", wk), ("v", wv)):
            w_sb[name] = xw_pool.tile(
                [P, DT, NW], mm_dt, tag=f"w{name}", name=f"w{name}"
            )
            nc.sync.dma_start(
                w_sb[name][:], wh[:].rearrange("(t p) n -> p t n", p=P)
            )
        xT_sb = xw_pool.tile([P, DT, S], mm_dt, tag="xT")
        for t in range(DT):
            nc.sync.dma_start(xT_sb[:, t, :], xT[t * P : (t + 1) * P, :])

        def proj_qk(pt, pool):
            # Q^T / K^T for head-pair pt: [n, s] layout, pair-stacked
            for ic in range(NIB):
                for wname, dst in (("q", qt_sb), ("k", kt_sb)):
                    ps = pool.tile([P, IB], F32, tag="of", name="ps")
                    for t in range(DT):
                        nc.tensor.matmul(
                            ps[:],
                            w_sb[wname][:, t, pt * P : (pt + 1) * P],
                            xT_sb[:, t, ic * IB : (ic + 1) * IB],
                            start=(t == 0),
                            stop=(t == DT - 1),
                        )
                    nc.vector.tensor_copy(
                        dst[pt][:, ic * IB : (ic + 1) * IB], ps[:]
                    )

        def proj_v(pool):
            # V: natural [s, n] layout, all heads, with ones column
            for st in range(ST):
                ps = pool.tile([P, NW], F32, tag="of", name="ps")
                for t in range(DT):
                    nc.tensor.matmul(
                        ps[:],
                        xT_sb[:, t, st * P : (st + 1) * P],
                        w_sb["v"][:, t, :],
                        start=(t == 0),
                        stop=(t == DT - 1),
                    )
                nc.vector.tensor_copy(
                    v_sb[:, st, :, 0:KS],
                    ps[:].rearrange("p (h k) -> p h k", k=KS),
                )

        def attention(pr, after_ib=None):
            # causal attention + output projection for head pair pr
            # (core heads 2*pr and 2*pr+1), scores row-packed via
            # tile_position so both heads' K=64 matmuls share the PE array
            for ib in range(NIB):
                if after_ib is not None and ib in after_ib:
                    after_ib[ib]()
                nl = (ib + 1) * LPB
                o_ps = [
                    po.tile([KS + 1, IB], F32, tag="of", name=f"o{pr}_{ib}_{hh}")
                    for hh in range(2)
                ]
                for lt in range(nl):
                    # causal: columns [0, off) of this i-block are fully
                    # masked for key tile lt; compute only the suffix
                    off = max(0, (lt - ib * LPB)) * P
                    st_ps = pst.tile([P, 2, IB], F32, tag="st", name="st")
                    for hh in range(2):
                        nc.tensor.matmul(
                            st_ps[:, hh, off:IB],
                            kt_sb[pr][hh * KS : (hh + 1) * KS, lt * P : (lt + 1) * P],
                            qt_sb[pr][
                                hh * KS : (hh + 1) * KS,
                                ib * IB + off : (ib + 1) * IB,
                            ],
                            start=True,
                            stop=True,
                            tile_position=(hh * KS, 0),
                        )
                    pe = pexp_pool.tile([P, 2, IB], BF16, tag="pe", name="pe")
                    nc.scalar.activation(
                        pe[:, :, off:IB], st_ps[:, :, off:IB], EXP, scale=0.125
                    )
                    if lt >= ib * LPB:  # diagonal 128-block -> triangular mask
                        for hh in range(2):
                            nc.vector.tensor_mul(
                                pe[:, hh, off : off + P],
                                pe[:, hh, off : off + P],
                                mask_sb[:],
                            )
                    for hh in range(2):
                        nc.tensor.matmul(
                            o_ps[hh][:, off:IB],
                            v_sb[:, lt, 2 * pr + hh, :],
                            pe[:, hh, off:IB],
                            start=(lt == 0),
                            stop=(lt == nl - 1),
                        )
                for hh in range(2):
                    h = 2 * pr + hh
                    # bf16 rows for the projection matmul, f32 Z row for
                    # exact normalization on the host
                    o_bf = osb_pool.tile([KS, IB], BF16, tag="o_bf", name="o_bf")
                    nc.vector.tensor_copy(o_bf[:], o_ps[hh][0:KS, :])
                    z_sb = osb_pool.tile([KS + 1, IB], F32, tag="z_sb", name="z_sb")
                    nc.vector.tensor_copy(
                        z_sb[KS : KS + 1, :], o_ps[hh][KS : KS + 1, :]
                    )
                    nc.sync.dma_start(
                        z[h, ib * IB : (ib + 1) * IB], z_sb[KS : KS + 1, :]
                    )
                    f_ps = po.tile([KS, IB], F32, tag="of", name="f_ps")
                    nc.tensor.matmul(
                        f_ps[:], wkern_sb[:, h, :], o_bf[:],
                        start=True, stop=True,
                    )
                    nc.vector.tensor_copy(
                        outT_sb[:, h, ib * IB : (ib + 1) * IB], f_ps[:]
                    )
            nc.sync.dma_start(
                outT[:].rearrange("h k s -> k h s")[:, 2 * pr : 2 * pr + 2, :],
                outT_sb[:, 2 * pr : 2 * pr + 2, :],
            )

        # PE warmup: dependency-free matmuls on zeroed scratch so the HAM
        # clock gate reaches 8/8 during the input-DMA lead-in, before real
        # matmuls (which otherwise run the whole projection phase at 1.2 GHz)
        warm_in = const_pool.tile([P, IB], BF16)
        nc.vector.memset(warm_in[:], 0.0)

        with tc.tile_pool(name="pproj", bufs=2, space=bass.MemorySpace.PSUM) as pp:
            for _ in range(45):
                w_ps = pp.tile([P, IB], F32, tag="of", name="w_ps")
                nc.tensor.matmul(
                    w_ps[:], warm_in[:, 0:P], warm_in[:], start=True, stop=True
                )
            proj_qk(0, pp)
            proj_v(pp)
        pst = ctx.enter_context(
            tc.tile_pool(name="pst", bufs=2, space=bass.MemorySpace.PSUM)
        )
        po = ctx.enter_context(
            tc.tile_pool(name="po", bufs=4, space=bass.MemorySpace.PSUM)
        )
        attention(0)
        proj_qk(1, po)  # emitted after attention(0): fills PE gaps during it
        attention(1)

    nc.compile()
    return nc


def make_masks():
    # triangular [P, P]: within a diagonal 128-block keep j >= p
    j = np.arange(P)[None, :]
    p = np.arange(P)[:, None]
    return (j >= p).astype(NP_BF16)


def make_in_maps(inputs):
    x = np.asarray(inputs["x"], np.float32)
    Wq = np.asarray(inputs["Wq"], np.float32)
    Wk = np.asarray(inputs["Wk"], np.float32)
    Wv = np.asarray(inputs["Wv"], np.float32)
    kern = np.asarray(inputs["kernel"], np.float32)

    masks = make_masks()
    kern3 = kern.reshape(KS, H, KS)  # [k, h, j]
    in_maps = []
    for c in range(NCORES):
        b, hs = c // CORES_PER_B, (c % CORES_PER_B) * NH
        in_maps.append(
            {
                "xT": x[b].T.astype(NP_BF16),
                "wq": Wq[:, :, hs : hs + NH].transpose(0, 2, 1).reshape(D, NW)
                .astype(NP_BF16),
                "wk": Wk[:, :, hs : hs + NH].transpose(0, 2, 1).reshape(D, NW)
                .astype(NP_BF16),
                "wv": Wv[:, :, hs : hs + NH].transpose(0, 2, 1).reshape(D, NW)
                .astype(NP_BF16),
                "wkern": kern3[:, hs : hs + NH, :].transpose(1, 0, 2)
                .astype(NP_BF16),
                "masks": masks,
            }
        )
    return in_maps


def gather_output(results):
    out = np.zeros((B, S, KS), np.float32)
    for c in range(NCORES):
        b = c // CORES_PER_B
        oT = np.asarray(results[c]["outT"], np.float32)  # [NH, KS, S]
        zz = np.asarray(results[c]["z"], np.float32)     # [NH, S]
        out[b] += (oT / zz[:, None, :]).sum(axis=0).T
    return out


_NC_CACHE = {}


def get_nc():
    if "nc" not in _NC_CACHE:
        _NC_CACHE["nc"] = build_nc()
    return _NC_CACHE["nc"]


def run_hw(inputs, trace=False, **kw):
    from concourse.bass_utils import run_bass_kernel_spmd

    nc = get_nc()
    in_maps = make_in_maps(inputs)
    res = run_bass_kernel_spmd(
        nc, in_maps, list(range(NCORES)), trace=trace, **kw
    )
    return gather_output(res.results), res


def kernel(**inputs) -> np.ndarray:
    out, _ = run_hw(inputs, trace=False)
    return out



# revision 11
# speedup vs baseline: 1.1015x; 1.1015x over previous
"""Trainium2 Bass kernel for nn_MultiHeadAttention_56375740727430.

Causal multi-head attention, B=2 S=2048 D=1024 H=16 KS=64, followed by an
output projection `heads @ kernel`.

Sharding: pure data/head parallel over 8 cores — core c handles batch c//4
and 4 heads (c%4)*4 ... +4.  Each core computes Q^T/K^T (head-pair-stacked,
transposed layout), V (natural layout, with an appended ones-column so the
softmax denominator Z falls out of the attention matmul), causal scores ->
exp -> (P@V | Z) -> per-head output projection, all unnormalized.  The host
divides by Z, sums head contributions and batch-partials, and transposes.

Performance structure:
- fp8e4 DoubleRow (K=256/instruction) for the QKV projections and for
  off-diagonal P@V pairs; exp writes e4m3 directly with a ln(8) bias that
  cancels in the host-side Z normalization.
- Hybrid precision: softmax averaging shrinks fp8 noise by ~1/sqrt(row
  support), so fp8 error only survives in early query rows.  Everything
  feeding rows i<512 (projection i-block 0, V tiles 0-3, attention block
  ib=0) therefore runs in bf16; W is host-prescaled by 64 in both dtypes
  (folded into the exp scale / V copy) to keep fp8 W out of e4m3's
  subnormal range.
- Scores stay bf16 (fp8 gains nothing at K=64; keeps exp input accurate);
  the two heads of a pair run concurrently via PE row tiling.
- x is DMA'd in (t, i-block) chunks and emission interleaves projections
  with attention blocks so exp (the scalar-engine bottleneck) starts early
  and runs back-to-back.
"""

import sys

sys.path.insert(0, "/opt/trn_rl_repo")

import math
import os
from contextlib import ExitStack

import ml_dtypes
import numpy as np

import concourse.bass as bass
import concourse.bacc as bacc
import concourse.mybir as mybir
import concourse.tile as tile

B, S, D = 2, 2048, 1024
H, KS = 16, 64

P = 128            # partitions
NCORES = 8
CORES_PER_B = NCORES // B          # 4
NH = H // CORES_PER_B              # heads per core = 4
NW = NH * KS                       # per-core projection width = 256
DT = D // P                        # d-tiles = 8
ST = S // P                        # s/l-tiles = 16
IB = 512                           # query block
NIB = S // IB                      # 4
LPB = IB // P                      # l-tiles per query block = 4

PROJ_FP8 = os.environ.get("K_PROJ_FP8", "1") == "1"  # QKV proj fp8 (ic>0)
PV_FP8 = os.environ.get("K_PV_FP8", "1") == "1"      # P@V fp8 (ib>0)
W_SCALE = 64.0                     # host prescale on Wq/Wk/Wv
PE_BIAS = math.log(8.0)            # exp bias on fp8 blocks (cancels via Z)
KSP = 68                           # padded V row count in fp8 mode (64+z+pad)

F32 = mybir.dt.float32
BF16 = mybir.dt.bfloat16
FP8 = mybir.dt.float8e4
NP_BF16 = ml_dtypes.bfloat16
NP_FP8 = ml_dtypes.float8_e4m3
EXP = mybir.ActivationFunctionType.Exp
DR = mybir.MatmulPerfMode.DoubleRow

EXP_SCALE = 0.125 / (W_SCALE * W_SCALE)


def build_nc():
    nc = bacc.Bacc()

    # bf16 x/weights feed the i<512 (bf16) blocks; fp8 copies feed the rest
    xT0 = nc.declare_dram_parameter("xT0", [D, S], BF16, isOutput=False)
    xT = nc.declare_dram_parameter("xT", [D, S], FP8, isOutput=False)
    wq = nc.declare_dram_parameter("wq", [D, NW], BF16, isOutput=False)
    wk = nc.declare_dram_parameter("wk", [D, NW], BF16, isOutput=False)
    wv = nc.declare_dram_parameter("wv", [D, NW], BF16, isOutput=False)
    wkern = nc.declare_dram_parameter("wkern", [NH, KS, KS], BF16, isOutput=False)
    masks = nc.declare_dram_parameter("masks", [P, P], BF16, isOutput=False)
    outT = nc.declare_dram_parameter("outT", [NH, KS, S], F32, isOutput=True)
    z = nc.declare_dram_parameter("z", [NH, S], F32, isOutput=True)

    with tile.TileContext(nc) as tc, ExitStack() as ctx:
        const_pool = ctx.enter_context(tc.tile_pool(name="const", bufs=1))
        qkv_pool = ctx.enter_context(tc.tile_pool(name="qkv", bufs=1))
        out_pool = ctx.enter_context(tc.tile_pool(name="outp", bufs=1))
        xw_pool = ctx.enter_context(tc.tile_pool(name="xw", bufs=1))
        pexp_pool = ctx.enter_context(tc.tile_pool(name="pexp", bufs=6))
        osb_pool = ctx.enter_context(tc.tile_pool(name="osb", bufs=4))
        pp = ctx.enter_context(
            tc.tile_pool(name="pp", bufs=2, space=bass.MemorySpace.PSUM)
        )
        pst = ctx.enter_context(
            tc.tile_pool(name="pst", bufs=2, space=bass.MemorySpace.PSUM)
        )
        po = ctx.enter_context(
            tc.tile_pool(name="po", bufs=1, space=bass.MemorySpace.PSUM)
        )

        # PE warmup: dependency-free matmuls on zeroed scratch so the HAM
        # clock gate reaches 8/8 during the input-DMA lead-in
        warm_in = const_pool.tile([P, P], BF16)
        nc.vector.memset(warm_in[:], 0.0)
        for wi in range(30):
            w_ps = pp.tile([P, IB], F32, tag="of", name=f"w_ps{wi}")
            nc.tensor.matmul(
                w_ps[:, 0:P], warm_in[:], warm_in[:], start=True, stop=True
            )

        wkern_sb = const_pool.tile([KS, NH, KS], BF16)
        nc.sync.dma_start(wkern_sb[:], wkern[:].rearrange("h k j -> k h j"))
        mask_sb = const_pool.tile([P, P], BF16)
        nc.sync.dma_start(mask_sb[:], masks[:])
        bias_sb = const_pool.tile([P, 1], F32)
        nc.vector.memset(bias_sb[:], PE_BIAS)

        w_sb, w8_sb = {}, {}
        for name, wh in (("q", wq), ("k", wk), ("v", wv)):
            w_sb[name] = xw_pool.tile(
                [P, DT, NW], BF16, tag=f"w{name}", name=f"w{name}"
            )
            nc.sync.dma_start(
                w_sb[name][:], wh[:].rearrange("(t p) n -> p t n", p=P)
            )
            if PROJ_FP8:
                w8_sb[name] = xw_pool.tile(
                    [P, DT, NW], FP8, tag=f"w8{name}", name=f"w8{name}"
                )
                nc.vector.tensor_copy(w8_sb[name][:], w_sb[name][:])
        # bf16 x covers only the i<512 block in fp8 mode, all of x otherwise
        XB = IB if PROJ_FP8 else S
        xb_sb = xw_pool.tile([P, DT, XB], BF16, tag="xb")
        for ic in range(XB // IB):
            for t in range(DT):
                nc.sync.dma_start(
                    xb_sb[:, t, ic * IB : (ic + 1) * IB],
                    xT0[t * P : (t + 1) * P, ic * IB : (ic + 1) * IB],
                )
        # fp8 x in (t, i-block) chunks, i-block major; the i<512 chunk is
        # only needed in bf16 (nothing reads fp8 x there)
        if PROJ_FP8:
            xT_sb = xw_pool.tile([P, DT, S], FP8, tag="xT")
            for ic in range(1, NIB):
                for t in range(DT):
                    nc.sync.dma_start(
                        xT_sb[:, t, ic * IB : (ic + 1) * IB],
                        xT[t * P : (t + 1) * P, ic * IB : (ic + 1) * IB],
                    )

        qt_sb = [
            qkv_pool.tile([P, S], BF16, tag=f"qt{i}", name=f"qt{i}") for i in range(2)
        ]
        kt_sb = [
            qkv_pool.tile([P, S], BF16, tag=f"kt{i}", name=f"kt{i}") for i in range(2)
        ]
        # bf16 V (ones col at 64) for the ib=0 attention block
        v0_sb = qkv_pool.tile([P, LPB, NH, KS + 1], BF16, tag="v0")
        nc.vector.memset(v0_sb[:, :, :, KS], 1.0)
        if PV_FP8:
            # fp8 V in DoubleRow-friendly (l-tile-pair, parity) layout
            v_sb = qkv_pool.tile([P, ST // 2, 2, NH, KSP], FP8, tag="v")
            nc.vector.memset(v_sb[:, :, :, :, KS], 1.0)
            nc.vector.memset(v_sb[:, :, :, :, KS + 1 : KSP], 0.0)
        else:
            v_sb = qkv_pool.tile([P, ST, NH, KS + 1], BF16, tag="v")
            nc.vector.memset(v_sb[:, :, :, KS], 1.0)
        outT_sb = out_pool.tile([KS, NH, S], F32)

        def mm_acc(ps, fp8, lhsT_of, rhs_of):
            """Full-D contraction: fp8 DoubleRow (4x K=256) or bf16 (8x)."""
            if fp8:
                for u in range(DT // 2):
                    nc.tensor.matmul(
                        ps, lhsT_of(2 * u, 2), rhs_of(2 * u, 2),
                        start=(u == 0), stop=(u == DT // 2 - 1),
                        perf_mode=DR,
                    )
            else:
                for t in range(DT):
                    nc.tensor.matmul(
                        ps, lhsT_of(t, 1), rhs_of(t, 1),
                        start=(t == 0), stop=(t == DT - 1),
                    )

        def proj_qk(pt, ic):
            # Q^T / K^T for head-pair pt, i-block ic: [n, s] layout
            fp8 = PROJ_FP8 and ic > 0
            wt, xt = (w8_sb, xT_sb) if fp8 else (w_sb, xb_sb)
            x0 = ic * IB if (fp8 or not PROJ_FP8) else 0  # xb holds ic=0 only
            for wname, dst in (("q", qt_sb), ("k", kt_sb)):
                ps = pp.tile([P, IB], F32, tag="of", name="ps")
                mm_acc(
                    ps[:], fp8,
                    lambda t, m, w=wname: wt[w][:, t : t + m, pt * P : (pt + 1) * P],
                    lambda t, m: xt[:, t : t + m, x0 : x0 + IB],
                )
                nc.vector.tensor_copy(dst[pt][:, ic * IB : (ic + 1) * IB], ps[:])

        def proj_v(st):
            # V: natural [s, n] layout, all heads, with ones column
            fp8 = PROJ_FP8 and st >= LPB
            wt, xt = (w8_sb, xT_sb) if fp8 else (w_sb, xb_sb)
            x0 = st * P  # st < 4 lies inside xb's i<512 window in fp8 mode
            ps = pp.tile([P, NW], F32, tag="of", name="ps")
            mm_acc(
                ps[:], fp8,
                lambda t, m: xt[:, t : t + m, x0 : x0 + P],
                lambda t, m: wt["v"][:, t : t + m, :],
            )
            src = ps[:].rearrange("p (h k) -> p h k", k=KS)
            if st < LPB:
                nc.vector.tensor_scalar_mul(
                    v0_sb[:, st, :, 0:KS], src, 1.0 / W_SCALE
                )
            if PV_FP8:
                dst = v_sb[:, st // 2, st % 2, :, 0:KS]
            else:
                dst = v_sb[:, st, :, 0:KS]
            nc.vector.tensor_scalar_mul(dst, src, 1.0 / W_SCALE)

        def attention_ib(pr, ib):
            # causal attention + output projection for head pair pr, i-block
            # ib; scores row-packed via tile_position so both heads' K=64
            # matmuls run concurrently on the PE array
            fp8 = PV_FP8 and ib > 0
            nl = (ib + 1) * LPB
            ndiag = ib * LPB  # l-tiles before the diagonal block (off == 0)
            o_ps = [
                po.tile([KSP, IB], F32, tag=f"o{hh}", name=f"o{pr}_{ib}_{hh}")
                for hh in range(2)
            ]
            pe_t = None
            for lt in range(nl):
                off = max(0, (lt - ndiag)) * P
                st_ps = pst.tile([P, 2, IB], F32, tag="st", name="st")
                for hh in range(2):
                    nc.tensor.matmul(
                        st_ps[:, hh, off:IB],
                        kt_sb[pr][hh * KS : (hh + 1) * KS, lt * P : (lt + 1) * P],
                        qt_sb[pr][
                            hh * KS : (hh + 1) * KS,
                            ib * IB + off : (ib + 1) * IB,
                        ],
                        start=True,
                        stop=True,
                        tile_position=(hh * KS, 0),
                    )
                if fp8:
                    if lt % 2 == 0:
                        pe_t = pexp_pool.tile(
                            [P, 2, 2, IB], FP8, tag="pe", name="pe"
                        )
                    pe_sl = pe_t[:, :, lt % 2, :]
                    nc.scalar.activation(
                        pe_sl[:, :, off:IB], st_ps[:, :, off:IB], EXP,
                        scale=EXP_SCALE, bias=bias_sb[:],
                    )
                else:
                    pe_t = pexp_pool.tile([P, 2, IB], BF16, tag="pe0", name="pe")
                    pe_sl = pe_t[:, :, :]
                    nc.scalar.activation(
                        pe_sl[:, :, off:IB], st_ps[:, :, off:IB], EXP,
                        scale=EXP_SCALE,
                    )
                if lt >= ndiag:  # diagonal 128-block -> triangular mask
                    for hh in range(2):
                        nc.vector.tensor_mul(
                            pe_sl[:, hh, off : off + P],
                            pe_sl[:, hh, off : off + P],
                            mask_sb[:],
                        )
                # P@V accumulation
                if fp8:
                    if lt < ndiag and lt % 2 == 1:
                        # completed off-diagonal pair: DoubleRow, K=256
                        for hh in range(2):
                            nc.tensor.matmul(
                                o_ps[hh][:],
                                v_sb[:, lt // 2, :, 2 * pr + hh, :],
                                pe_t[:, hh, :, :],
                                start=(lt == 1), stop=(lt == nl - 1),
                                perf_mode=DR,
                            )
                    elif lt >= ndiag:
                        for hh in range(2):
                            nc.tensor.matmul(
                                o_ps[hh][:, off:IB],
                                v_sb[:, lt // 2, lt % 2, 2 * pr + hh, :],
                                pe_t[:, hh, lt % 2, off:IB],
                                start=(lt == 0), stop=(lt == nl - 1),
                            )
                else:
                    for hh in range(2):
                        vsl = (
                            v0_sb[:, lt, 2 * pr + hh, :]
                            if ib == 0
                            else v_sb[:, lt, 2 * pr + hh, :]
                        )
                        nc.tensor.matmul(
                            o_ps[hh][0 : KS + 1, off:IB],
                            vsl,
                            pe_t[:, hh, off:IB],
                            start=(lt == 0), stop=(lt == nl - 1),
                        )
            for hh in range(2):
                h = 2 * pr + hh
                # bf16 rows for the projection matmul, f32 Z row for
                # exact normalization on the host
                o_bf = osb_pool.tile([KS, IB], BF16, tag="o_bf", name="o_bf")
                nc.vector.tensor_copy(o_bf[:], o_ps[hh][0:KS, :])
                z_sb = osb_pool.tile([1, IB], F32, tag="z_sb", name="z_sb")
                nc.vector.tensor_copy(z_sb[:], o_ps[hh][KS : KS + 1, :])
                nc.sync.dma_start(z[h, ib * IB : (ib + 1) * IB], z_sb[:])
                f_ps = pp.tile([KS, IB], F32, tag="of", name="f_ps")
                nc.tensor.matmul(
                    f_ps[:], wkern_sb[:, h, :], o_bf[:], start=True, stop=True
                )
                nc.vector.tensor_copy(
                    outT_sb[:, h, ib * IB : (ib + 1) * IB], f_ps[:]
                )
            nc.sync.dma_start(
                outT[:].rearrange("h k s -> k h s")[
                    :, 2 * pr : 2 * pr + 2, ib * IB : (ib + 1) * IB
                ],
                outT_sb[:, 2 * pr : 2 * pr + 2, ib * IB : (ib + 1) * IB],
            )

        # pair 0: interleave projections with attention blocks so exp (the
        # scalar-engine bottleneck) starts early and never starves
        for ib in range(NIB):
            proj_qk(0, ib)
            for st in range(ib * LPB, (ib + 1) * LPB):
                proj_v(st)
            attention_ib(0, ib)
        for ic in range(NIB):
            proj_qk(1, ic)
        for ib in range(NIB):
            attention_ib(1, ib)

    nc.compile()
    return nc


def make_masks():
    # triangular [P, P]: within a diagonal 128-block keep j >= p
    j = np.arange(P)[None, :]
    p = np.arange(P)[:, None]
    return (j >= p).astype(NP_BF16)


def make_in_maps(inputs):
    x = np.asarray(inputs["x"], np.float32)
    Wq = np.asarray(inputs["Wq"], np.float32)
    Wk = np.asarray(inputs["Wk"], np.float32)
    Wv = np.asarray(inputs["Wv"], np.float32)
    kern = np.asarray(inputs["kernel"], np.float32)

    masks = make_masks()
    kern3 = kern.reshape(KS, H, KS)  # [k, h, j]

    def packw(W, hs):
        Wp = W[:, :, hs : hs + NH].transpose(0, 2, 1).reshape(D, NW) * W_SCALE
        return Wp.astype(NP_BF16)

    in_maps = []
    for c in range(NCORES):
        b, hs = c // CORES_PER_B, (c % CORES_PER_B) * NH
        xb = x[b].T  # [D, S]
        in_maps.append(
            {
                "xT0": xb.astype(NP_BF16),
                "xT": np.clip(xb, -240, 240).astype(NP_FP8),
                "wq": packw(Wq, hs),
                "wk": packw(Wk, hs),
                "wv": packw(Wv, hs),
                "wkern": kern3[:, hs : hs + NH, :].transpose(1, 0, 2)
                .astype(NP_BF16),
                "masks": masks,
            }
        )
    return in_maps


def gather_output(results):
    out = np.zeros((B, S, KS), np.float32)
    for c in range(NCORES):
        b = c // CORES_PER_B
        oT = np.asarray(results[c]["outT"], np.float32)  # [NH, KS, S]
        zz = np.asarray(results[c]["z"], np.float32)     # [NH, S]
        out[b] += (oT / zz[:, None, :]).sum(axis=0).T
    return out


_NC_CACHE = {}


def get_nc():
    if "nc" not in _NC_CACHE:
        _NC_CACHE["nc"] = build_nc()
    return _NC_CACHE["nc"]


def run_hw(inputs, trace=False, **kw):
    from concourse.bass_utils import run_bass_kernel_spmd

    nc = get_nc()
    in_maps = make_in_maps(inputs)
    res = run_bass_kernel_spmd(
        nc, in_maps, list(range(NCORES)), trace=trace, **kw
    )
    return gather_output(res.results), res


def kernel(**inputs) -> np.ndarray:
    out, _ = run_hw(inputs, trace=False)
    return out
